# revision 19
# baseline (speedup 1.0000x reference)
"""Fused AttnBlock kernel for 8 Trainium2 NeuronCores (v2).

Problem: q = LN_head(x1 @ wq + bq), k = LN_head(x2 @ wk + bk), v = x2 @ wv + bv,
out = softmax(q k^T / sqrt(D)) v, with B=4, N=2048, C=1024, H=16, D=64.

Sharding: data-parallel over batch (4) x tensor-parallel over head groups (2).
Each core handles one (batch, head-group) pair fully locally; no collectives.

v2 key ideas (vs the v1 baseline preserved below as build_legacy):
  - LN mean subtraction is folded into the WEIGHTS on the host: wq/wk get
    their per-head column means subtracted, so q/k come out of the projection
    already centered. Only the 1/std scale remains of LayerNorm.
  - k's scale is folded into the softmax exp: exp((rk[m]/8) * s[m,n]) uses
    ACT's per-partition scale operand - zero extra work for k's LN. (Scores
    are bilinear, so per-row/per-column scales commute with the matmul; the
    mean-cross-terms vanish because the other side is exactly zero-sum.)
  - q's scale is a per-head ACT Identity apply (scale only, no bias).
  - variance via DVE square + grouped reduce_sum ([128,8,64] -> [128,8] in
    one op) instead of 16x bn_stats/bn_aggr per tile.
  - score matmuls run with K=128 instead of K=64: K<=64 matmuls stream moving
    columns at HALF rate (427 vs 216 ns per 512 cols, measured). k is
    rearranged into a block-diagonal stationary (two 64-wide m-chunks on the
    two partition halves, zeros elsewhere) and q is duplicated across both
    partition halves, so one [128,128]x[128,512] matmul computes 128 m
    positions at full rate.
  - softmax denominators (from the PV ones-column) are batch-reciprocal'd in
    two [16,512] DVE ops instead of 16 [64,512] ones.
  - input DMAs ride the SP queue; the kblk scatter / qdup mirror DMAs ride
    the ACT hwdge queue so they don't serialize behind the 11MB input load.
"""

import os
import sys

for _p in ("/opt/trn_rl_repo",):
    if _p not in sys.path:
        sys.path.insert(0, _p)

import ml_dtypes
import numpy as np

import concourse.bass as bass
import concourse.mybir as mybir
import concourse.tile as tile
from concourse.bass_utils import run_bass_kernel_spmd

F32 = mybir.dt.float32
BF16 = mybir.dt.bfloat16

B = 4
NSEQ = 2048
DIM = 1024
NHEADS = 16
HDIM = 64
EPS = 1e-5

NCORES = 8
LAST_RESULTS = None
HG = 8            # heads per core
JW = HG * HDIM    # 512 output channels per core
KT = DIM // 128   # 8 contraction tiles for the projections
NT = NSEQ // 128  # 16 n-tiles


def split_multi_waits(nc, maxw=1):
    # TRN2 instructions carry a single sem-wait slot; this walrus build rejects
    # more. Tile's exit drain accumulates one wait per engine/DMA queue, so
    # hoist the excess onto injected NoOps just before the offending inst.
    for bb in nc.main_func.blocks:
        new_insts = []
        for inst in bb.instructions:
            si = inst.sync_info
            if si is not None and si.on_wait and len(si.on_wait) > maxw:
                waits = list(si.on_wait)
                extra, keep = waits[:-maxw], waits[-maxw:]
                for ci in range(0, len(extra), maxw):
                    nop = mybir.InstNoOp(
                        name=nc.get_next_instruction_name(), ins=[], outs=[],
                        sync_info=mybir.SyncInfo(
                            on_wait=extra[ci:ci + maxw], on_update=[]),
                    )
                    nop.engine = inst.engine
                    new_insts.append(nop)
                    nc.register_instruction(nop, overwrite=True)
                inst.sync_info = mybir.SyncInfo(
                    on_wait=keep, on_update=list(si.on_update))
            new_insts.append(inst)
        bb.instructions[:] = new_insts


def build_v2(n_seq=NSEQ):
    nt_n = n_seq // 128
    sw = min(1024, n_seq)
    nblk = n_seq // sw
    nch = sw // 512

    nc = bass.Bass()
    x1t = nc.dram_tensor("x1t", [DIM, n_seq], BF16, kind="ExternalInput")
    x2t = nc.dram_tensor("x2t", [DIM, n_seq], BF16, kind="ExternalInput")
    wq_d = nc.dram_tensor("wq", [DIM, JW], BF16, kind="ExternalInput")
    wk_d = nc.dram_tensor("wk", [DIM, JW], BF16, kind="ExternalInput")
    wv_d = nc.dram_tensor("wv", [DIM, JW], BF16, kind="ExternalInput")
    eye_d = nc.dram_tensor("eye", [128, 128], BF16, kind="ExternalInput")
    out_d = nc.dram_tensor("outT", [JW, n_seq], F32, kind="ExternalOutput")

    with tile.TileContext(nc) as tc:
        with tc.tile_pool(name="persist", bufs=1) as persist:
            qdup = persist.tile([128, HG, n_seq], BF16)
            kblk = persist.tile([128, HG, nt_n, 128], BF16)
            vA = persist.tile([128, nt_n, HG, HDIM + 1], BF16)
            rk8 = persist.tile([128, nt_n, HG], F32)   # (1/8) * rstd_k
            eye_sb = persist.tile([128, 128], BF16)
            eps_sb = persist.tile([128, 1], F32)
            eps64_sb = persist.tile([128, 1], F32)
            nc.vector.memset(eps_sb, EPS)
            nc.vector.memset(eps64_sb, HDIM * EPS)
            nc.vector.memset(vA[:, :, :, HDIM:HDIM + 1], 1.0)
            nc.vector.memset(kblk, 0.0)
            nc.sync.dma_start(out=eye_sb, in_=eye_d[:, :])

            # x2sb + wv live in an outer pool: the v projection is interleaved
            # into the attention stream (phase B), where the PE has slack
            # under the ACT-bound exp stream.
            with tc.tile_pool(name="wout", bufs=1) as wout:
                x2sb = wout.tile([128, KT, n_seq], BF16, name="x2sb")
                w_v = wout.tile([128, KT, JW], BF16, name="w_v")

                # ------- phase A: q/k projections + LN + transposes -------
                with tc.tile_pool(name="wpool", bufs=1) as wpool, \
                     tc.tile_pool(name="lnb", bufs=4) as ln_pool, \
                     tc.tile_pool(name="stats", bufs=3) as st_pool, \
                     tc.tile_pool(name="pps", bufs=6, space="PSUM") as proj_ps, \
                     tc.tile_pool(name="tps", bufs=2, space="PSUM") as tp_ps:

                    x1sb = wpool.tile([128, KT, n_seq], BF16, name="x1sb")
                    w_q = wpool.tile([128, KT, JW], BF16, name="w_q")
                    w_k = wpool.tile([128, KT, JW], BF16, name="w_k")

                    # k phase runs first: wk + x2 on the SP queue; x1/wq ride
                    # the ACT hwdge queue in parallel
                    x1r = x1t.rearrange("(kt p) n -> p kt n", p=128)
                    x2r = x2t.rearrange("(kt p) n -> p kt n", p=128)
                    wqr = wq_d.rearrange("(kt p) j -> p kt j", p=128)
                    for ct in range(KT):
                        nc.sync.dma_start(out=w_q[:, ct, :],
                                          in_=wqr[:, ct, :])
                    nc.sync.dma_start(out=x1sb[:, :, 0:128],
                                      in_=x1r[:, :, 0:128])
                    nc.scalar.dma_start(
                        out=w_k, in_=wk_d.rearrange("(kt p) j -> p kt j",
                                                    p=128))
                    nc.scalar.dma_start(
                        out=w_v, in_=wv_d.rearrange("(kt p) j -> p kt j",
                                                    p=128))
                    nc.sync.dma_start(out=x1sb[:, :, 128:256],
                                      in_=x1r[:, :, 128:256])
                    nc.scalar.dma_start(out=x2sb[:, :, 0:256],
                                        in_=x2r[:, :, 0:256])
                    for xi in range(1, 8):
                        xs = slice(xi * 256, (xi + 1) * 256)
                        nc.sync.dma_start(out=x1sb[:, :, xs],
                                          in_=x1r[:, :, xs])
                        nc.scalar.dma_start(out=x2sb[:, :, xs],
                                            in_=x2r[:, :, xs])

                    def project(ps, xc, w):
                        for ct in range(KT):
                            nc.tensor.matmul(ps, xc[:, ct, :], w[:, ct, :],
                                             start=(ct == 0),
                                             stop=(ct == KT - 1))

                    def stats_rstd(src_bf16, scale, bias_tile, dst_rstd):
                        # dst = 1/sqrt(scale*sum_d(src^2) + bias) per head.
                        # src is the bf16 SBUF copy of the projection: the
                        # square runs on DVE in 2x bf16 mode and ACT only
                        # does the tiny per-tile Sqrt. (A PSUM-sourced DVE
                        # square is illegal: one PSUM read port.)
                        sq = st_pool.tile([128, HG, 66], BF16, name="sq",
                                          tag="sq")
                        sg = src_bf16.rearrange("p (h d) -> p h d", h=HG)
                        nc.vector.tensor_mul(out=sq[:, :, 0:HDIM], in0=sg,
                                             in1=sg)
                        ssum = st_pool.tile([128, HG], F32, name="ssum",
                                            tag="ssum")
                        nc.vector.reduce_sum(out=ssum, in_=sq[:, :, 0:HDIM],
                                             axis=mybir.AxisListType.X)
                        std = st_pool.tile([128, HG], F32, name="std",
                                           tag="std")
                        nc.scalar.activation(
                            out=std, in_=ssum,
                            func=mybir.ActivationFunctionType.Sqrt,
                            bias=bias_tile, scale=float(scale))
                        nc.vector.reciprocal(out=dst_rstd, in_=std)

                    # ---- interleaved q/k/v projections per n-tile (chains
                    # on ACT/DVE overlap the next tensor's matmuls) ----
                    with tc.tile_pool(name="ktp", bufs=1) as kt_pool:
                        kT = kt_pool.tile([128, 4, n_seq], BF16, name="kT")

                        def k_drains(src, nt):
                            nsl = slice(nt * 128, (nt + 1) * 128)
                            for jt in range(4):
                                tp = tp_ps.tile([128, 128], BF16, name="tp",
                                                tag="tp")
                                nc.tensor.transpose(
                                    tp, src[:, jt * 128:(jt + 1) * 128],
                                    eye_sb)
                                if jt % 2 == 0:
                                    nc.scalar.copy(out=kT[:, jt, nsl],
                                                   in_=tp)
                                else:
                                    nc.vector.tensor_copy(out=kT[:, jt, nsl],
                                                          in_=tp)

                        def q_drains(src, nt):
                            nsl = slice(nt * 128, (nt + 1) * 128)
                            for jt in range(4):
                                tp = tp_ps.tile([128, 128], BF16, name="tp",
                                                tag="tp")
                                nc.tensor.transpose(
                                    tp, src[:, jt * 128:(jt + 1) * 128],
                                    eye_sb)
                                if jt % 2 == 0:
                                    nc.vector.tensor_copy(
                                        out=qdup[0:64, 2 * jt, nsl],
                                        in_=tp[0:64, :])
                                    nc.scalar.copy(
                                        out=qdup[64:128, 2 * jt + 1, nsl],
                                        in_=tp[64:128, :])
                                else:
                                    nc.scalar.copy(
                                        out=qdup[0:64, 2 * jt, nsl],
                                        in_=tp[0:64, :])
                                    nc.vector.tensor_copy(
                                        out=qdup[64:128, 2 * jt + 1, nsl],
                                        in_=tp[64:128, :])

                        pending = []
                        for nt in range(nt_n):
                            nsl = slice(nt * 128, (nt + 1) * 128)
                            # q
                            ps = proj_ps.tile([128, JW], F32, name="ps",
                                              tag="ps")
                            project(ps, x1sb[:, :, nsl], w_q)
                            qsb = ln_pool.tile([128, JW], BF16, name="qsb",
                                               tag="lnb")
                            nc.vector.tensor_copy(out=qsb, in_=ps)
                            rstd = st_pool.tile([128, HG], F32, name="rstd",
                                                tag="rstd")
                            stats_rstd(qsb, 1.0 / HDIM, eps_sb, rstd)
                            ln = ln_pool.tile([128, JW], BF16, name="ln",
                                              tag="lnb")
                            for h in range(HG):
                                if h % 2 == 0:
                                    nc.scalar.activation(
                                        out=ln[:, h * HDIM:(h + 1) * HDIM],
                                        in_=qsb[:, h * HDIM:(h + 1) * HDIM],
                                        func=(mybir.ActivationFunctionType
                                              .Identity),
                                        scale=rstd[:, h:h + 1])
                                else:
                                    nc.vector.tensor_scalar(
                                        out=ln[:, h * HDIM:(h + 1) * HDIM],
                                        in0=qsb[:, h * HDIM:(h + 1) * HDIM],
                                        scalar1=rstd[:, h:h + 1],
                                        scalar2=None,
                                        op0=mybir.AluOpType.mult)
                            pending.append((q_drains, ln, nt))
                            # k
                            ps = proj_ps.tile([128, JW], F32, name="ps",
                                              tag="ps")
                            project(ps, x2sb[:, :, nsl], w_k)
                            ksb = ln_pool.tile([128, JW], BF16, name="ksb",
                                               tag="lnb")
                            nc.vector.tensor_copy(out=ksb, in_=ps)
                            stats_rstd(ksb, 1.0, eps64_sb, rk8[:, nt, :])
                            pending.append((k_drains, ksb, nt))
                            # v
                            ps = proj_ps.tile([128, JW], F32, name="ps",
                                              tag="ps")
                            project(ps, x2sb[:, :, nsl], w_v)
                            nc.vector.tensor_copy(
                                out=vA[:, nt, :, 0:HDIM],
                                in_=ps.rearrange("p (h d) -> p h d", h=HG))
                            while len(pending) > 4:
                                fn, src, pnt = pending.pop(0)
                                fn(src, pnt)
                        for fn, src, pnt in pending:
                            fn(src, pnt)

                        # block-diagonal scatter, per head (DMA AP balancing
                        # caps at 3 dims). SP queue: free after input load.
                        for h in range(HG):
                            src = kT[64 * (h % 2):64 * (h % 2) + 64,
                                     h // 2, :]
                            srcg = src.rearrange("p (mi u) -> p mi u", u=64)
                            nc.sync.dma_start(out=kblk[0:64, h, :, 0:64],
                                              in_=srcg[:, 0::2, :])
                            nc.sync.dma_start(
                                out=kblk[64:128, h, :, 64:128],
                                in_=srcg[:, 1::2, :])

                    # mirror the missing partition half of each head,
                    # per head in consumption order
                    for h in range(HG):
                        have = 64 * (h % 2)
                        miss = 64 - have
                        nc.sync.dma_start(out=qdup[miss:miss + 64, h, :],
                                          in_=qdup[have:have + 64, h, :])

                # -------------------- phase B: attention --------------------
                # v's projection rides inside head 0 / block 0: one n-tile of
                # v per mi slot, always >= LAG slots ahead of the PV that
                # consumes it.
                with tc.tile_pool(name="sps", bufs=1, space="PSUM") as s_ps, \
                     tc.tile_pool(name="pvps", bufs=2, space="PSUM") as pv_ps, \
                     tc.tile_pool(name="psb", bufs=4) as p_pool, \
                     tc.tile_pool(name="nrm", bufs=4) as n_pool, \
                     tc.tile_pool(name="ob0", bufs=6) as ob0_pool, \
                     tc.tile_pool(name="stg", bufs=4) as stg_pool, \
                     tc.tile_pool(name="dnp", bufs=2) as den_pool, \
                     tc.tile_pool(name="dsc", bufs=4, space="DRAM") as dram_pool:
                    for h in range(HG):
                        for blk in range(nblk):
                            pvs = [pv_ps.tile([65, 512], F32, name=f"pv{c2}",
                                              tag="pv") for c2 in range(nch)]
                            LAG = 2
                            p_tiles = {}
                            for mi in range(nt_n + LAG):
                                if mi < nt_n:
                                    stag = "s" if mi % 3 < 2 else "s3"
                                    s = s_ps.tile([128, sw], F32, name="s",
                                                  tag=stag,
                                                  bufs=(2 if stag == "s"
                                                        else 1))
                                    for c2 in range(nch):
                                        n0 = blk * sw + c2 * 512
                                        nc.tensor.matmul(
                                            s[:, c2 * 512:(c2 + 1) * 512],
                                            kblk[:, h, mi, :],
                                            qdup[:, h, n0:n0 + 512],
                                            start=True, stop=True)
                                    p = p_pool.tile([128, sw], BF16, name="p",
                                                    tag="p")
                                    nc.scalar.activation(
                                        out=p, in_=s,
                                        func=mybir.ActivationFunctionType.Exp,
                                        scale=rk8[:, mi, h:h + 1])
                                    p_tiles[mi] = p
                                if mi >= LAG:
                                    m = mi - LAG
                                    p = p_tiles.pop(m)
                                    for c2 in range(nch):
                                        nc.tensor.matmul(
                                            pvs[c2], vA[:, m, h, :],
                                            p[:, c2 * 512:(c2 + 1) * 512],
                                            start=(m == 0),
                                            stop=(m == nt_n - 1))
                            # drain numerators + denominator rows, then
                            # normalize this block while the next computes.
                            # den rows land in a [2,512] pool tile (pool
                            # tiles start at partition 0 -> aligned recip)
                            dens_b = den_pool.tile([2, 512], F32,
                                                   name="dens", tag="dens")
                            osb0s = []
                            for c2 in range(nch):
                                stg = stg_pool.tile([1, 512], F32, name="stg",
                                                    tag="stg")
                                nc.vector.tensor_copy(out=stg,
                                                      in_=pvs[c2][64:65, :])
                                nc.sync.dma_start(
                                    out=dens_b[c2:c2 + 1, :], in_=stg)
                                osb0 = ob0_pool.tile([64, 512], F32,
                                                     name="osb0", tag="osb0")
                                nc.vector.tensor_copy(out=osb0,
                                                      in_=pvs[c2][0:64, :])
                                osb0s.append(osb0)
                            denr_b = den_pool.tile([2, 512], F32, name="denr",
                                                   tag="denr")
                            nc.vector.reciprocal(out=denr_b, in_=dens_b)
                            for c2 in range(nch):
                                dscr = dram_pool.tile([512], F32, name="dscr",
                                                      tag="dscr")
                                nc.sync.dma_start(
                                    out=dscr, in_=denr_b[c2:c2 + 1, :])
                                denb = n_pool.tile([64, 512], F32,
                                                   name="denb", tag="denb")
                                dscr_b = bass.AP(
                                    tensor=dscr.tensor, offset=dscr.offset,
                                    ap=[[0, 64]] + list(dscr.ap))
                                nc.sync.dma_start(out=denb, in_=dscr_b)
                                osb = n_pool.tile([64, 512], F32, name="osb",
                                                  tag="osb")
                                nc.vector.tensor_mul(
                                    out=osb, in0=osb0s[c2], in1=denb)
                                n0 = blk * sw + c2 * 512
                                nc.sync.dma_start(
                                    out=out_d[h * HDIM:(h + 1) * HDIM,
                                              n0:n0 + 512],
                                    in_=osb)
    split_multi_waits(nc)
    return nc


def shard_inputs_v2(x1, x2, wq, wkv, n_seq=NSEQ):
    bf16 = ml_dtypes.bfloat16
    eye = np.eye(128, dtype=bf16)
    # center per-head column groups so projections emit zero-mean q/k
    wq_c = wq.copy()
    wk_c = wkv[:, :DIM].copy()
    for h in range(NHEADS):
        g = slice(h * HDIM, (h + 1) * HDIM)
        wq_c[:, g] -= wq_c[:, g].mean(axis=1, keepdims=True)
        wk_c[:, g] -= wk_c[:, g].mean(axis=1, keepdims=True)
    wv = wkv[:, DIM:]
    in_maps = []
    for core in range(NCORES):
        b, g = divmod(core, 2)
        jsl = slice(g * JW, (g + 1) * JW)
        m = {
            "x1t": np.ascontiguousarray(x1[b, :n_seq].T.astype(bf16)),
            "x2t": np.ascontiguousarray(x2[b, :n_seq].T.astype(bf16)),
            "wq": np.ascontiguousarray(wq_c[:, jsl].astype(bf16)),
            "wk": np.ascontiguousarray(wk_c[:, jsl].astype(bf16)),
            "wv": np.ascontiguousarray(wv[:, jsl].astype(bf16)),
            "eye": eye,
        }
        in_maps.append(m)
    return in_maps


# --------------------------------------------------------------------------
# v1 baseline kernel, kept as the generic fallback for nonzero biases or
# non-identity gamma/beta (the staged problem has neither).
# --------------------------------------------------------------------------

def build_legacy(n_seq=NSEQ, has_bq=False, has_bkv=False, has_gbq=False,
                 has_gbk=False):
    nt_n = n_seq // 128
    sw = min(1024, n_seq)
    nblk = n_seq // sw
    nch = sw // 512
    scale = 1.0 / np.sqrt(HDIM)

    nc = bass.Bass()
    x1t = nc.dram_tensor("x1t", [DIM, n_seq], BF16, kind="ExternalInput")
    x2t = nc.dram_tensor("x2t", [DIM, n_seq], BF16, kind="ExternalInput")
    wq_d = nc.dram_tensor("wq", [DIM, JW], BF16, kind="ExternalInput")
    wk_d = nc.dram_tensor("wk", [DIM, JW], BF16, kind="ExternalInput")
    wv_d = nc.dram_tensor("wv", [DIM, JW], BF16, kind="ExternalInput")
    eye_d = nc.dram_tensor("eye", [128, 128], BF16, kind="ExternalInput")
    if has_bq:
        bq_d = nc.dram_tensor("bq", [JW], F32, kind="ExternalInput")
    if has_bkv:
        bk_d = nc.dram_tensor("bk", [JW], F32, kind="ExternalInput")
        bv_d = nc.dram_tensor("bv", [JW], F32, kind="ExternalInput")
    if has_gbq:
        gq_d = nc.dram_tensor("gq", [JW], F32, kind="ExternalInput")
        betq_d = nc.dram_tensor("betq", [JW], F32, kind="ExternalInput")
    if has_gbk:
        gk_d = nc.dram_tensor("gk", [JW], F32, kind="ExternalInput")
        betk_d = nc.dram_tensor("betk", [JW], F32, kind="ExternalInput")
    out_d = nc.dram_tensor("outT", [JW, n_seq], F32, kind="ExternalOutput")

    def bcast_from_dram(pool, vec_d, name):
        t = pool.tile([128, JW], F32, name=name)
        src = bass.AP(tensor=vec_d.tensor, offset=vec_d.offset,
                      ap=[[0, 128]] + list(vec_d.ap))
        nc.sync.dma_start(out=t, in_=src)
        return t

    with tile.TileContext(nc) as tc:
        with tc.tile_pool(name="persist", bufs=1) as persist:
            qT = persist.tile([128, 4, n_seq], BF16)
            kT = persist.tile([128, 4, n_seq], BF16)
            vA = persist.tile([128, nt_n, HG, HDIM + 1], BF16)
            eye_sb = persist.tile([128, 128], BF16)
            eps_sb = persist.tile([128, 1], F32)
            nc.sync.dma_start(out=eye_sb, in_=eye_d[:, :])
            nc.vector.memset(eps_sb, EPS)
            nc.vector.memset(vA[:, :, :, HDIM:HDIM + 1], 1.0)

            bqb = bcast_from_dram(persist, bq_d[:], "bqb") if has_bq else None
            bkb = bcast_from_dram(persist, bk_d[:], "bkb") if has_bkv else None
            bvb = bcast_from_dram(persist, bv_d[:], "bvb") if has_bkv else None
            gqb = bcast_from_dram(persist, gq_d[:], "gqb") if has_gbq else None
            btqb = bcast_from_dram(persist, betq_d[:], "btqb") if has_gbq \
                else None
            gkb = bcast_from_dram(persist, gk_d[:], "gkb") if has_gbk else None
            btkb = bcast_from_dram(persist, betk_d[:], "btkb") if has_gbk \
                else None

            with tc.tile_pool(name="wpool", bufs=1) as wpool, \
                 tc.tile_pool(name="lnb", bufs=6) as ln_pool, \
                 tc.tile_pool(name="stats", bufs=6) as st_pool, \
                 tc.tile_pool(name="pps", bufs=6, space="PSUM") as proj_ps, \
                 tc.tile_pool(name="tps", bufs=2, space="PSUM") as tp_ps:

                w_sb = {}
                for nm in ("q", "k", "v"):
                    w_sb[nm] = wpool.tile([128, KT, JW], BF16, name=f"w_{nm}")
                x1sb = wpool.tile([128, KT, n_seq], BF16, name="x1sb")
                x2sb = wpool.tile([128, KT, n_seq], BF16, name="x2sb")
                xq = n_seq // 4
                x1r = x1t.rearrange("(kt p) n -> p kt n", p=128)
                x2r = x2t.rearrange("(kt p) n -> p kt n", p=128)
                nc.sync.dma_start(
                    out=w_sb["q"],
                    in_=wq_d.rearrange("(kt p) j -> p kt j", p=128))
                nc.sync.dma_start(out=x1sb[:, :, 0:128], in_=x1r[:, :, 0:128])
                nc.sync.dma_start(
                    out=w_sb["k"],
                    in_=wk_d.rearrange("(kt p) j -> p kt j", p=128))
                nc.sync.dma_start(out=x2sb[:, :, 0:128], in_=x2r[:, :, 0:128])
                nc.sync.dma_start(
                    out=w_sb["v"],
                    in_=wv_d.rearrange("(kt p) j -> p kt j", p=128))
                if xq > 128:
                    nc.sync.dma_start(out=x1sb[:, :, 128:xq],
                                      in_=x1r[:, :, 128:xq])
                    nc.sync.dma_start(out=x2sb[:, :, 128:xq],
                                      in_=x2r[:, :, 128:xq])
                for xi in range(1, 4):
                    xs = slice(xi * xq, (xi + 1) * xq)
                    nc.sync.dma_start(out=x1sb[:, :, xs], in_=x1r[:, :, xs])
                    nc.sync.dma_start(out=x2sb[:, :, xs], in_=x2r[:, :, xs])

                def layernorm_into(psum, dst, bias_b, gb, bb_):
                    if bias_b is not None:
                        src = ln_pool.tile([128, JW], F32, name="biased",
                                           tag="biased")
                        nc.vector.tensor_add(out=src, in0=psum, in1=bias_b)
                    else:
                        src = psum
                    stats = st_pool.tile([128, HG, 6], F32, name="stats")
                    for h in range(HG):
                        nc.vector.bn_stats(
                            out=stats[:, h, :],
                            in_=src[:, h * HDIM:(h + 1) * HDIM])
                    mv = st_pool.tile([128, HG, 2], F32, name="mv")
                    for h in range(HG):
                        nc.vector.bn_aggr(out=mv[:, h, :], in_=stats[:, h, :])
                    std = st_pool.tile([128, HG], F32, name="std")
                    nc.scalar.activation(
                        out=std, in_=mv[:, :, 1],
                        func=mybir.ActivationFunctionType.Sqrt,
                        bias=eps_sb, scale=1.0)
                    rstd = st_pool.tile([128, HG], F32, name="rstd")
                    nc.vector.reciprocal(out=rstd, in_=std)
                    negmr = st_pool.tile([128, HG], F32, name="negmr")
                    nc.vector.tensor_mul(out=negmr, in0=mv[:, :, 0],
                                         in1=rstd)
                    nc.vector.tensor_scalar(
                        out=negmr, in0=negmr, scalar1=-1.0, scalar2=None,
                        op0=mybir.AluOpType.mult)
                    for h in range(HG):
                        nc.scalar.activation(
                            out=dst[:, h * HDIM:(h + 1) * HDIM],
                            in_=src[:, h * HDIM:(h + 1) * HDIM],
                            func=mybir.ActivationFunctionType.Identity,
                            bias=negmr[:, h:h + 1], scale=rstd[:, h:h + 1])
                    if gb is not None:
                        nc.vector.tensor_mul(out=dst, in0=dst, in1=gb)
                        nc.vector.tensor_add(out=dst, in0=dst, in1=bb_)

                def emit_transposes(ln, dstT, nt):
                    nsl = slice(nt * 128, (nt + 1) * 128)
                    for jt in range(4):
                        tp = tp_ps.tile([128, 128], BF16, name="tp", tag="tp")
                        nc.tensor.transpose(
                            tp, ln[:, jt * 128:(jt + 1) * 128], eye_sb)
                        if jt % 2 == 0:
                            nc.vector.tensor_copy(out=dstT[:, jt, nsl], in_=tp)
                        else:
                            nc.scalar.copy(out=dstT[:, jt, nsl], in_=tp)

                pending = []
                for nt in range(nt_n):
                    nsl = slice(nt * 128, (nt + 1) * 128)
                    x1c = x1sb[:, :, nsl]
                    x2c = x2sb[:, :, nsl]

                    for nm, xc, dstT, bias_b, gb, bb_ in (
                        ("q", x1c, qT, bqb, gqb, btqb),
                        ("k", x2c, kT, bkb, gkb, btkb),
                    ):
                        ps = proj_ps.tile([128, JW], F32, name="ps", tag="ps")
                        for ct in range(KT):
                            nc.tensor.matmul(
                                ps, xc[:, ct, :], w_sb[nm][:, ct, :],
                                start=(ct == 0), stop=(ct == KT - 1))
                        ln = ln_pool.tile([128, JW], BF16, name="ln", tag="ln")
                        layernorm_into(ps, ln, bias_b, gb, bb_)
                        pending.append((ln, dstT, nt))

                    ps = proj_ps.tile([128, JW], F32, name="ps", tag="ps")
                    for ct in range(KT):
                        nc.tensor.matmul(
                            ps, x2c[:, ct, :], w_sb["v"][:, ct, :],
                            start=(ct == 0), stop=(ct == KT - 1))
                    psg = ps.rearrange("p (h d) -> p h d", h=HG)
                    if bvb is not None:
                        nc.vector.tensor_add(
                            out=vA[:, nt, :, 0:HDIM], in0=psg,
                            in1=bvb.rearrange("p (h d) -> p h d", h=HG))
                    else:
                        nc.vector.tensor_copy(out=vA[:, nt, :, 0:HDIM],
                                              in_=psg)
                    while len(pending) > 2:
                        emit_transposes(*pending.pop(0))
                for args in pending:
                    emit_transposes(*args)

            with tc.tile_pool(name="sps", bufs=2, space="PSUM") as s_ps, \
                 tc.tile_pool(name="pvps", bufs=2, space="PSUM") as pv_ps, \
                 tc.tile_pool(name="psb", bufs=3) as p_pool, \
                 tc.tile_pool(name="nrm", bufs=3) as n_pool, \
                 tc.tile_pool(name="dsc", bufs=4, space="DRAM") as dram_pool:
                for h in range(HG):
                    pt, bp = divmod(h, 2)
                    prows = slice(bp * 64, (bp + 1) * 64)
                    kTh = kT[prows, pt, :]
                    qTh = qT[prows, pt, :]
                    for blk in range(nblk):
                        pvs = [pv_ps.tile([65, 512], F32, name=f"pv{c2}",
                                          tag="pv") for c2 in range(nch)]
                        LAG = 2
                        p_tiles = {}
                        for mi in range(nt_n + LAG):
                            if mi < nt_n:
                                s = s_ps.tile([128, sw], F32, name="s",
                                              tag="s", bufs=LAG + 1)
                                for c2 in range(nch):
                                    n0 = blk * sw + c2 * 512
                                    nc.tensor.matmul(
                                        s[:, c2 * 512:(c2 + 1) * 512],
                                        kTh[:, mi * 128:(mi + 1) * 128],
                                        qTh[:, n0:n0 + 512],
                                        start=True, stop=True)
                                p = p_pool.tile([128, sw], BF16, name="p",
                                                tag="p", bufs=LAG + 2)
                                nc.scalar.activation(
                                    out=p, in_=s,
                                    func=mybir.ActivationFunctionType.Exp,
                                    scale=float(scale))
                                p_tiles[mi] = p
                            if mi >= LAG:
                                m = mi - LAG
                                p = p_tiles.pop(m)
                                for c2 in range(nch):
                                    nc.tensor.matmul(
                                        pvs[c2], vA[:, m, h, :],
                                        p[:, c2 * 512:(c2 + 1) * 512],
                                        start=(m == 0), stop=(m == nt_n - 1))
                        dens = n_pool.tile([64, 512], F32, name="dens",
                                           tag="dens")
                        nc.vector.memset(dens, 1.0)
                        osb0s = []
                        for c2 in range(nch):
                            osb0 = n_pool.tile([64, 512], F32, name="osb0",
                                               tag="osb0", bufs=4)
                            nc.vector.tensor_copy(out=osb0,
                                                  in_=pvs[c2][0:64, :])
                            nc.scalar.copy(out=dens[32 * c2:32 * c2 + 1, :],
                                           in_=pvs[c2][64:65, :])
                            osb0s.append(osb0)
                        denr = n_pool.tile([64, 512], F32, name="denr",
                                           tag="denr")
                        nc.vector.reciprocal(out=denr, in_=dens)
                        for c2 in range(nch):
                            dscr = dram_pool.tile([512], F32, name="dscr",
                                                  tag="dscr")
                            nc.sync.dma_start(
                                out=dscr, in_=denr[32 * c2:32 * c2 + 1, :])
                            denb_sb = n_pool.tile([64, 512], F32,
                                                  name="denb_sb",
                                                  tag="denb_sb")
                            dscr_b = bass.AP(tensor=dscr.tensor,
                                             offset=dscr.offset,
                                             ap=[[0, 64]] + list(dscr.ap))
                            nc.sync.dma_start(out=denb_sb, in_=dscr_b)
                            osb = n_pool.tile([64, 512], F32, name="osb",
                                              tag="osb")
                            nc.vector.tensor_mul(
                                out=osb, in0=osb0s[c2], in1=denb_sb)
                            n0 = blk * sw + c2 * 512
                            nc.sync.dma_start(
                                out=out_d[h * HDIM:(h + 1) * HDIM,
                                          n0:n0 + 512],
                                in_=osb)
    split_multi_waits(nc)
    return nc


def shard_inputs_legacy(x1, x2, wq, bq, wkv, bkv, gamma_q, beta_q, gamma_k,
                        beta_k, flags, n_seq=NSEQ):
    has_bq, has_bkv, has_gbq, has_gbk = flags
    bf16 = ml_dtypes.bfloat16
    eye = np.eye(128, dtype=bf16)
    in_maps = []
    for core in range(NCORES):
        b, g = divmod(core, 2)
        jsl = slice(g * JW, (g + 1) * JW)
        m = {
            "x1t": np.ascontiguousarray(x1[b, :n_seq].T.astype(bf16)),
            "x2t": np.ascontiguousarray(x2[b, :n_seq].T.astype(bf16)),
            "wq": np.ascontiguousarray(wq[:, jsl].astype(bf16)),
            "wk": np.ascontiguousarray(wkv[:, jsl].astype(bf16)),
            "wv": np.ascontiguousarray(
                wkv[:, DIM + g * JW:DIM + (g + 1) * JW].astype(bf16)),
            "eye": eye,
        }
        if has_bq:
            m["bq"] = np.ascontiguousarray(bq[jsl])
        if has_bkv:
            m["bk"] = np.ascontiguousarray(bkv[jsl])
            m["bv"] = np.ascontiguousarray(
                bkv[DIM + g * JW:DIM + (g + 1) * JW])
        if has_gbq:
            m["gq"] = np.tile(gamma_q, HG).astype(np.float32)
            m["betq"] = np.tile(beta_q, HG).astype(np.float32)
        if has_gbk:
            m["gk"] = np.tile(gamma_k, HG).astype(np.float32)
            m["betk"] = np.tile(beta_k, HG).astype(np.float32)
        in_maps.append(m)
    return in_maps


def kernel(x1, x2, wq, bq, wkv, bkv, gamma_q, beta_q, gamma_k, beta_k):
    x1 = np.asarray(x1, dtype=np.float32)
    x2 = np.asarray(x2, dtype=np.float32)
    wq = np.asarray(wq, dtype=np.float32)
    bq = np.asarray(bq, dtype=np.float32)
    wkv = np.asarray(wkv, dtype=np.float32)
    bkv = np.asarray(bkv, dtype=np.float32)
    gamma_q = np.asarray(gamma_q, dtype=np.float32)
    beta_q = np.asarray(beta_q, dtype=np.float32)
    gamma_k = np.asarray(gamma_k, dtype=np.float32)
    beta_k = np.asarray(beta_k, dtype=np.float32)

    flags = (
        bool(np.any(bq)),
        bool(np.any(bkv)),
        not (np.all(gamma_q == 1.0) and np.all(beta_q == 0.0)),
        not (np.all(gamma_k == 1.0) and np.all(beta_k == 0.0)),
    )
    trace = bool(int(os.environ.get("KERNEL_TRACE", "0")))
    global LAST_RESULTS
    if any(flags):
        nc = build_legacy(NSEQ, *flags)
        in_maps = shard_inputs_legacy(x1, x2, wq, bq, wkv, bkv, gamma_q,
                                      beta_q, gamma_k, beta_k, flags)
    else:
        nc = build_v2(NSEQ)
        in_maps = shard_inputs_v2(x1, x2, wq, wkv)
    res = run_bass_kernel_spmd(nc, in_maps, core_ids=list(range(NCORES)),
                               trace=trace)
    LAST_RESULTS = res
    out = np.empty((B, NSEQ, DIM), dtype=np.float32)
    for core in range(NCORES):
        b, g = divmod(core, 2)
        out[b, :, g * JW:(g + 1) * JW] = res.results[core]["outT"].T
    return out
